# revision 4
# baseline (speedup 1.0000x reference)
"""DeeperGCN Trainium2 kernel (8 NeuronCores, SPMD).

Strategy:
  - Nodes padded to NPAD = 8*NSH and sharded by dst across 8 cores.
  - Per conv layer, per-node messages u = m*exp(t*m), w = exp(t*m)
    (m = relu(h)+eps) are computed shard-wise (feature-major), transposed to
    a node-major uv table [NPAD, 2H] bf16 and replicated to every core via
    AllGather (double-buffered across layers).
  - Edge aggregation: dma_gather of uv rows by src (int16 index windows),
    then a segmented sum over dst via TensorE matmuls with one-hot fp8
    R matrices (host-precomputed, streamed from HBM):
       psum[2H feat, 128 dst] += gathered_uv_chunk[128e,2Hf]^T @ R[128e,128d]
  - agg = num/den; MLP + LayerNorm evaluated feature-major; LN over features
    uses ones-matmul broadcast stats (no transposes).
Host preprocessing (edge bucketing, R matrices, transpose/cast of x) is
numpy; only device NEFF time counts.
"""
import numpy as np
import ml_dtypes
from dataclasses import dataclass, field

EPS_MSG = 1e-7
LN_EPS = 1e-5
NCORES = 8


@dataclass
class Cfg:
    N: int = 100000
    E: int = 1000000
    F_IN: int = 500
    H: int = 64
    C: int = 3
    L: int = 3
    NSH: int = 12544            # nodes/core, multiple of 128
    NSW: int = 4                # src windows
    SLOTS_G: int = 80           # gather-buffer chunk slots per super-chunk
    MCH: int = 448              # MLP node-chunk (<=512)

    @property
    def NPAD(self):
        return NCORES * self.NSH

    @property
    def NW(self):
        return self.NSH // 128

    @property
    def WSZ(self):
        assert self.NPAD % self.NSW == 0
        w = self.NPAD // self.NSW
        assert w <= 32768
        return w

    @property
    def HH(self):
        return 2 * self.H

    @property
    def FPAD(self):
        return ((self.F_IN + 127) // 128) * 128


CFG = Cfg()


def _cdiv(a, b):
    return (a + b - 1) // b


# --------------------------------------------------------------------------
# host: edge structures
# --------------------------------------------------------------------------

def build_edge_structs(edge_index, cfg=CFG):
    src = np.asarray(edge_index[0], np.int64)
    dst = np.asarray(edge_index[1], np.int64)
    NSH, NW, NSW, WSZ = cfg.NSH, cfg.NW, cfg.NSW, cfg.WSZ
    core = dst // NSH
    dstloc = dst % NSH
    w = dstloc // 128
    dcol = dstloc % 128
    s = src // WSZ
    srcloc = src % WSZ

    counts = np.zeros((NCORES, NW, NSW), np.int64)
    np.add.at(counts, (core, w, s), 1)
    PBc = _cdiv(counts, 128).max(axis=0)           # [NW, NSW]
    PBc = np.maximum(PBc, 1)                       # keep >=1 chunk per bucket

    win_chunks = PBc.sum(axis=1)
    groups, cur, cur_slots = [], [], 0
    for wi in range(NW):
        c = int(win_chunks[wi])
        if cur and cur_slots + c > cfg.SLOTS_G:
            groups.append(cur)
            cur, cur_slots = [], 0
        cur.append(wi)
        cur_slots += c
    if cur:
        groups.append(cur)

    IW = [int(PBc[:, si].sum()) * 128 for si in range(NSW)]
    CTOT = int(PBc.sum())

    # chunk (R) order: for g: for w in g: for s: chunks
    ch_off = np.zeros((NW, NSW), np.int64)
    ct = 0
    for g in groups:
        for wi in g:
            for si in range(NSW):
                ch_off[wi, si] = ct
                ct += int(PBc[wi, si])
    assert ct == CTOT
    # gather idx offset within src window si (group-major == natural order)
    gx_off = np.zeros((NW, NSW), np.int64)
    for si in range(NSW):
        o = 0
        for wi in range(NW):
            gx_off[wi, si] = o
            o += int(PBc[wi, si]) * 128
        assert o == IW[si]

    order = np.lexsort((w, s, core))
    src_s, core_s, s_s, w_s, dcol_s = (
        srcloc[order], core[order], s[order], w[order], dcol[order])

    gidx_all, rmat_all = [], []
    for k in range(NCORES):
        sel = core_s == k
        ks, kw, kdc, ksrc = s_s[sel], w_s[sel], dcol_s[sel], src_s[sel]
        gidx = [np.zeros(IW[si], np.int16) for si in range(NSW)]
        rmat = np.zeros((128, CTOT, 128), ml_dtypes.float8_e4m3)
        # bucket boundaries within this core's (sorted by s, w) edge list
        for si in range(NSW):
            insel = ks == si
            kwsi, kdcsi, ksrcsi = kw[insel], kdc[insel], ksrc[insel]
            bnd = np.searchsorted(kwsi, np.arange(NW + 1))
            for wi in range(NW):
                a, b = bnd[wi], bnd[wi + 1]
                n = b - a
                cap = int(PBc[wi, si]) * 128
                assert n <= cap
                o = gx_off[wi, si]
                gidx[si][o: o + n] = ksrcsi[a:b].astype(np.int16)
                j = np.arange(n)
                rmat[j % 128, ch_off[wi, si] + j // 128, kdcsi[a:b]] = 1.0
        gidx_all.append(np.concatenate(gidx))
        rmat_all.append(rmat)

    return dict(PBc=PBc, groups=groups, IW=IW, CTOT=CTOT, ch_off=ch_off,
                gx_off=gx_off, gidx=gidx_all, rmat=rmat_all)


# --------------------------------------------------------------------------
# device builder
# --------------------------------------------------------------------------

def build_nc(structs, cfg=CFG):
    import concourse.bass as bass
    import concourse.tile as tile
    from concourse import bacc, mybir
    from contextlib import ExitStack

    dt = mybir.dt
    PBc, groups, IW, CTOT = (structs["PBc"], structs["groups"],
                             structs["IW"], structs["CTOT"])
    ch_off, gx_off = structs["ch_off"], structs["gx_off"]
    NSH, NW, NSW, WSZ = cfg.NSH, cfg.NW, cfg.NSW, cfg.WSZ
    H, HH, C, L, FPAD = cfg.H, cfg.HH, cfg.C, cfg.L, cfg.FPAD
    MCH = cfg.MCH
    NMC = NSH // MCH
    assert NMC * MCH == NSH
    IWALL = sum(IW)
    IW_base = np.concatenate([[0], np.cumsum(IW)]).astype(np.int64)

    # params column map
    PCOL = {"encb": 0}
    nc_col = 1
    for l in range(L):
        for nm in ("b1", "g1", "be1", "b2", "ng", "nb", "t"):
            PCOL[(nm, l)] = nc_col
            nc_col += 1
    PCOL["linb"] = nc_col
    PCOL["eps"] = nc_col + 1
    NPCOL = nc_col + 2

    nc = bacc.Bacc("TRN2", num_swdge_queues=4)
    xt_d = nc.declare_dram_parameter("xt", [FPAD, NSH], dt.bfloat16, isOutput=False)
    encw_d = nc.declare_dram_parameter("encw", [128, FPAD // 128, H], dt.bfloat16, isOutput=False)
    w1_d = nc.declare_dram_parameter("w1", [H, L, HH], dt.bfloat16, isOutput=False)
    w2_d = nc.declare_dram_parameter("w2", [HH, L, H], dt.bfloat16, isOutput=False)
    linw_d = nc.declare_dram_parameter("linw", [H, C], dt.bfloat16, isOutput=False)
    ident_d = nc.declare_dram_parameter("ident", [128, 128], dt.bfloat16, isOutput=False)
    params_d = nc.declare_dram_parameter("params", [128, NPCOL], dt.float32, isOutput=False)
    gidx_d = nc.declare_dram_parameter("gidx", [128, IWALL // 16], dt.int16, isOutput=False)
    rmat_d = nc.declare_dram_parameter("rmat", [128, CTOT, 128], dt.float8e4, isOutput=False)
    outp_d = nc.declare_dram_parameter("outp", [C, NSH], dt.float32, isOutput=True)

    uvshard = nc.dram_tensor("uvshard", [NSH, HH], dt.bfloat16)
    tabs = [nc.dram_tensor(f"uvtab{i}", [cfg.NPAD, HH], dt.bfloat16,
                           addr_space="Shared") for i in range(2)]

    with tile.TileContext(nc) as tc, ExitStack() as ctx:
        const = ctx.enter_context(tc.tile_pool(name="const", bufs=1))
        sb_par = const.tile([128, NPCOL], dt.float32)
        nc.sync.dma_start(sb_par[:], params_d[:])
        sb_encw = const.tile([128, FPAD // 128, H], dt.bfloat16)
        nc.sync.dma_start(sb_encw[:], encw_d[:])
        sb_w1 = const.tile([H, L, HH], dt.bfloat16)
        nc.sync.dma_start(sb_w1[:], w1_d[:])
        sb_w2 = const.tile([HH, L, H], dt.bfloat16)
        nc.sync.dma_start(sb_w2[:], w2_d[:])
        sb_linw = const.tile([H, C], dt.bfloat16)
        nc.sync.dma_start(sb_linw[:], linw_d[:])
        sb_id = const.tile([128, 128], dt.bfloat16)
        nc.sync.dma_start(sb_id[:], ident_d[:])
        sb_o128 = const.tile([128, 128], dt.bfloat16)
        nc.vector.memset(sb_o128[:], 1.0 / 128)
        sb_o64 = const.tile([H, H], dt.bfloat16)
        nc.vector.memset(sb_o64[:], 1.0 / H)
        sb_gidx = const.tile([128, IWALL // 16], dt.int16)
        nc.sync.dma_start(sb_gidx[:], gidx_d[:])

        def pcol(key, rows=128):
            cidx = PCOL[key]
            return sb_par[0:rows, cidx: cidx + 1]

        master = ctx.enter_context(tc.tile_pool(name="master", bufs=1))
        hT = master.tile([H, NSH], dt.float32)
        numden = master.tile([HH, NSH], dt.float32)

        # ---------------- encoder: hT = (x @ enc_W + b)^T ----------------
        with tc.tile_pool(name="enc", bufs=3) as ep, \
             tc.tile_pool(name="encps", bufs=2, space="PSUM") as pp:
            for c in range(NMC):
                sl = slice(c * MCH, (c + 1) * MCH)
                xtile = ep.tile([128, FPAD // 128, MCH], dt.bfloat16)
                for fc in range(FPAD // 128):
                    nc.sync.dma_start(
                        xtile[:, fc, :], xt_d[fc * 128:(fc + 1) * 128, sl])
                ps = pp.tile([H, MCH], dt.float32)
                for fc in range(FPAD // 128):
                    nc.tensor.matmul(ps[:], sb_encw[:, fc, :], xtile[:, fc, :],
                                     start=(fc == 0), stop=(fc == FPAD // 128 - 1))
                nc.vector.tensor_scalar_add(hT[:, sl], ps[:], pcol("encb", H))

        # ---------------- conv layers ----------------
        conv_params = [0] + list(range(L))          # [0, 0, 1, 2]
        for conv, l in enumerate(conv_params):
            is_first = conv == 0
            tab = tabs[conv % 2]

            # U phase: uv2 [HH, NSH] bf16 (rows 0:H = u, rows H:HH = w)
            with tc.tile_pool(name="uvp", bufs=1) as uvp, \
                 tc.tile_pool(name="uvs", bufs=3) as us:
                uv2 = uvp.tile([HH, NSH], dt.bfloat16)
                UCH = MCH * 2
                for c in range(NSH // UCH):
                    sl = slice(c * UCH, (c + 1) * UCH)
                    h2 = us.tile([HH, UCH], dt.float32)
                    nc.gpsimd.dma_start(h2[0:H, :], hT[:, sl])
                    nc.gpsimd.dma_start(h2[H:HH, :], hT[:, sl])
                    m2 = us.tile([HH, UCH], dt.bfloat16)
                    nc.vector.tensor_scalar(
                        m2[:], h2[:], 0.0, EPS_MSG,
                        mybir.AluOpType.max, mybir.AluOpType.add)
                    # uv2 = exp(t*m2); then rows 0:H *= m2
                    nc.scalar.activation(uv2[:, sl], m2[:],
                                         mybir.ActivationFunctionType.Exp,
                                         scale=pcol(("t", l), HH))
                    nc.vector.tensor_mul(uv2[0:H, sl], m2[0:H, :], uv2[0:H, sl])
                # T phase: transpose to node-major -> uvshard
                with tc.tile_pool(name="tps", bufs=4, space="PSUM") as tpp, \
                     tc.tile_pool(name="tst", bufs=4) as tst:
                    for nt in range(NSH // 128):
                        tp = tpp.tile([128, HH], dt.bfloat16)
                        nc.tensor.transpose(
                            tp[:], uv2[:, nt * 128:(nt + 1) * 128], sb_id[:])
                        st = tst.tile([128, HH], dt.bfloat16)
                        if nt % 2 == 0:
                            nc.scalar.copy(st[:], tp[:])
                        else:
                            nc.vector.tensor_copy(st[:], tp[:])
                        nc.sync.dma_start(
                            uvshard[nt * 128:(nt + 1) * 128, :], st[:])

            # AllGather shard -> full table (double-buffered across convs)
            nc.gpsimd.collective_compute(
                "AllGather", mybir.AluOpType.bypass,
                replica_groups=[list(range(NCORES))],
                ins=[uvshard[:, :]], outs=[tab[:, :]])

            # G+S phase
            with tc.tile_pool(name="gb", bufs=2) as gp, \
                 tc.tile_pool(name="rb", bufs=2) as rp, \
                 tc.tile_pool(name="gsps", bufs=4, space="PSUM") as pp:
                ct_base = 0
                for g in groups:
                    slots_s = [int(PBc[g, si].sum()) for si in range(NSW)]
                    tot = sum(slots_s)
                    gbuf = gp.tile([128, tot, HH], dt.bfloat16, tag="gbuf")
                    off = 0
                    reg_off = []
                    for si in range(NSW):
                        nidx = slots_s[si] * 128
                        reg_off.append(off)
                        if nidx:
                            a = IW_base[si] + gx_off[g[0], si]
                            nc.gpsimd.dma_gather(
                                gbuf[:, off: off + slots_s[si], :],
                                tab[si * WSZ: (si + 1) * WSZ, :],
                                sb_gidx[:, a // 16: (a + nidx) // 16],
                                nidx, nidx, HH, single_packet=False,
                                queue_num=si)
                        off += slots_s[si]
                    rtile = rp.tile([128, tot, 128], dt.float8e4, tag="rt")
                    nc.sync.dma_start(
                        rtile[:], rmat_d[:, ct_base: ct_base + tot, :])
                    for wi in g:
                        ps = pp.tile([HH, 128], dt.float32)
                        nchw = int(PBc[wi].sum())
                        done = 0
                        for si in range(NSW):
                            for j in range(int(PBc[wi, si])):
                                slot = (reg_off[si]
                                        + int(gx_off[wi, si] - gx_off[g[0], si]) // 128
                                        + j)
                                ct = int(ch_off[wi, si]) - ct_base + j
                                nc.tensor.matmul(
                                    ps[:], gbuf[:, slot, :], rtile[:, ct, :],
                                    start=(done == 0), stop=(done == nchw - 1))
                                done += 1
                        wsl = slice(wi * 128, (wi + 1) * 128)
                        if wi % 2 == 0:
                            nc.scalar.copy(numden[:, wsl], ps[:])
                        else:
                            nc.vector.tensor_copy(numden[:, wsl], ps[:])
                    ct_base += tot

            # M phase: agg -> MLP -> LN -> residual, chunked over nodes
            with tc.tile_pool(name="mp", bufs=2) as mp, \
                 tc.tile_pool(name="mps", bufs=1, space="PSUM") as pp, \
                 tc.tile_pool(name="mps64", bufs=1, space="PSUM") as pp64:
                AF = mybir.ActivationFunctionType
                AL = mybir.AluOpType
                for c in range(NMC):
                    sl = slice(c * MCH, (c + 1) * MCH)
                    dn = mp.tile([H, MCH], dt.float32, tag="s1")
                    nc.vector.tensor_scalar_max(dn[:], numden[H:HH, sl], 1e-30)
                    rden = mp.tile([H, MCH], dt.float32, tag="s2")
                    nc.vector.reciprocal(rden[:], dn[:])
                    t1 = mp.tile([H, MCH], dt.float32, tag="s3")
                    nc.vector.tensor_mul(t1[:], numden[0:H, sl], rden[:])
                    rbf = mp.tile([H, MCH], dt.bfloat16, tag="s4")
                    nc.vector.tensor_add(rbf[:], t1[:], hT[:, sl])
                    ps1 = pp.tile([HH, MCH], dt.float32, tag="p1")
                    nc.tensor.matmul(ps1[:], sb_w1[:, l, :], rbf[:])
                    ybf = mp.tile([HH, MCH], dt.bfloat16, tag="s5")
                    nc.vector.tensor_scalar_add(ybf[:], ps1[:], pcol(("b1", l)))
                    sq = mp.tile([HH, MCH], dt.bfloat16, tag="s6")
                    nc.scalar.square(sq[:], ybf[:])
                    psmu = pp.tile([HH, MCH], dt.float32, tag="p2")
                    nc.tensor.matmul(psmu[:], sb_o128[:], ybf[:])
                    pss2 = pp.tile([HH, MCH], dt.float32, tag="p3")
                    nc.tensor.matmul(pss2[:], sb_o128[:], sq[:])
                    v1 = mp.tile([HH, MCH], dt.float32, tag="s7")
                    nc.scalar.square(v1[:], psmu[:])
                    nc.vector.tensor_sub(v1[:], pss2[:], v1[:])
                    v3 = mp.tile([HH, MCH], dt.float32, tag="s8")
                    nc.scalar.activation(v3[:], v1[:], AF.Sqrt, bias=pcol("eps", HH))
                    nc.vector.reciprocal(v3[:], v3[:])
                    d = mp.tile([HH, MCH], dt.float32, tag="s9")
                    nc.vector.tensor_sub(d[:], ybf[:], psmu[:])
                    nc.vector.tensor_mul(d[:], d[:], v3[:])
                    f_ = mp.tile([HH, MCH], dt.float32, tag="s10")
                    nc.vector.tensor_scalar(f_[:], d[:], pcol(("g1", l)),
                                            pcol(("be1", l)), AL.mult, AL.add)
                    h1 = mp.tile([HH, MCH], dt.bfloat16, tag="s11")
                    nc.scalar.activation(h1[:], f_[:], AF.Relu)
                    ps2 = pp64.tile([H, MCH], dt.float32, tag="p4")
                    nc.tensor.matmul(ps2[:], sb_w2[:, l, :], h1[:])
                    if is_first:
                        nc.vector.tensor_scalar_add(hT[:, sl], ps2[:],
                                                    pcol(("b2", l), H))
                    else:
                        cbf = mp.tile([H, MCH], dt.bfloat16, tag="s12")
                        nc.vector.tensor_scalar_add(cbf[:], ps2[:],
                                                    pcol(("b2", l), H))
                        sq2 = mp.tile([H, MCH], dt.bfloat16, tag="s13")
                        nc.scalar.square(sq2[:], cbf[:])
                        pmu = pp64.tile([H, MCH], dt.float32, tag="p5")
                        nc.tensor.matmul(pmu[:], sb_o64[:], cbf[:])
                        ps2b = pp64.tile([H, MCH], dt.float32, tag="p6")
                        nc.tensor.matmul(ps2b[:], sb_o64[:], sq2[:])
                        u1 = mp.tile([H, MCH], dt.float32, tag="s14")
                        nc.scalar.square(u1[:], pmu[:])
                        nc.vector.tensor_sub(u1[:], ps2b[:], u1[:])
                        u3 = mp.tile([H, MCH], dt.float32, tag="s15")
                        nc.scalar.activation(u3[:], u1[:], AF.Sqrt, bias=pcol("eps", H))
                        nc.vector.reciprocal(u3[:], u3[:])
                        dd = mp.tile([H, MCH], dt.float32, tag="s16")
                        nc.vector.tensor_sub(dd[:], cbf[:], pmu[:])
                        nc.vector.tensor_mul(dd[:], dd[:], u3[:])
                        ff = mp.tile([H, MCH], dt.float32, tag="s17")
                        nc.vector.tensor_scalar(ff[:], dd[:], pcol(("ng", l), H),
                                                pcol(("nb", l), H), AL.mult, AL.add)
                        cr = mp.tile([H, MCH], dt.float32, tag="s18")
                        nc.scalar.activation(cr[:], ff[:], AF.Relu)
                        nc.vector.tensor_add(hT[:, sl], hT[:, sl], cr[:])

        # ---------------- final head ----------------
        with tc.tile_pool(name="hd", bufs=3) as mp, \
             tc.tile_pool(name="hdps", bufs=1, space="PSUM") as pp:
            AF = mybir.ActivationFunctionType
            AL = mybir.AluOpType
            for c in range(NMC):
                sl = slice(c * MCH, (c + 1) * MCH)
                hbf = mp.tile([H, MCH], dt.bfloat16, tag="t1")
                nc.vector.tensor_copy(hbf[:], hT[:, sl])
                sq = mp.tile([H, MCH], dt.bfloat16, tag="t2")
                nc.scalar.square(sq[:], hbf[:])
                pmu = pp.tile([H, MCH], dt.float32, tag="q1")
                nc.tensor.matmul(pmu[:], sb_o64[:], hbf[:])
                ps2b = pp.tile([H, MCH], dt.float32, tag="q2")
                nc.tensor.matmul(ps2b[:], sb_o64[:], sq[:])
                u1 = mp.tile([H, MCH], dt.float32, tag="t3")
                nc.scalar.square(u1[:], pmu[:])
                nc.vector.tensor_sub(u1[:], ps2b[:], u1[:])
                u3 = mp.tile([H, MCH], dt.float32, tag="t4")
                nc.scalar.activation(u3[:], u1[:], AF.Sqrt, bias=pcol("eps", H))
                nc.vector.reciprocal(u3[:], u3[:])
                dd = mp.tile([H, MCH], dt.float32, tag="t5")
                nc.vector.tensor_sub(dd[:], hT[:, sl], pmu[:])
                nc.vector.tensor_mul(dd[:], dd[:], u3[:])
                ff = mp.tile([H, MCH], dt.float32, tag="t6")
                nc.vector.tensor_scalar(ff[:], dd[:], pcol(("ng", 0), H),
                                        pcol(("nb", 0), H), AL.mult, AL.add)
                fbf = mp.tile([H, MCH], dt.bfloat16, tag="t7")
                nc.scalar.activation(fbf[:], ff[:], AF.Relu)
                pso = pp.tile([C, MCH], dt.float32, tag="q3")
                nc.tensor.matmul(pso[:], sb_linw[:], fbf[:])
                ot = mp.tile([C, MCH], dt.float32, tag="t8")
                nc.vector.tensor_scalar_add(ot[:], pso[:], pcol("linb", C))
                nc.sync.dma_start(outp_d[:, sl], ot[:])

    nc.compile()
    return nc, NPCOL, PCOL


# --------------------------------------------------------------------------
# host: input packing
# --------------------------------------------------------------------------

def pack_inputs(inputs, structs, NPCOL, PCOL, cfg=CFG):
    bf16 = ml_dtypes.bfloat16
    NSH, NPAD, FPAD = cfg.NSH, cfg.NPAD, cfg.FPAD
    H, HH, C, L = cfg.H, cfg.HH, cfg.C, cfg.L

    x = np.asarray(inputs["x"], np.float32)
    xp = np.zeros((NPAD, FPAD), np.float32)
    xp[: x.shape[0], : x.shape[1]] = x

    encw = np.zeros((FPAD, H), np.float32)
    encw[: cfg.F_IN] = np.asarray(inputs["enc_W"], np.float32)
    encw = np.ascontiguousarray(
        encw.reshape(FPAD // 128, 128, H).transpose(1, 0, 2)).astype(bf16)

    w1 = np.ascontiguousarray(
        np.asarray(inputs["W1"], np.float32).transpose(1, 0, 2)).astype(bf16)
    w2 = np.ascontiguousarray(
        np.asarray(inputs["W2"], np.float32).transpose(1, 0, 2)).astype(bf16)
    linw = np.asarray(inputs["lin_W"], np.float32).astype(bf16)
    ident = np.eye(128, dtype=bf16)

    params = np.zeros((128, NPCOL), np.float32)
    params[:H, PCOL["encb"]] = inputs["enc_b"]
    for l in range(L):
        params[:, PCOL[("b1", l)]] = inputs["b1"][l]
        params[:, PCOL[("g1", l)]] = inputs["g1"][l]
        params[:, PCOL[("be1", l)]] = inputs["be1"][l]
        params[:H, PCOL[("b2", l)]] = inputs["b2"][l]
        params[:H, PCOL[("ng", l)]] = inputs["ng"][l]
        params[:H, PCOL[("nb", l)]] = inputs["nb"][l]
        params[:, PCOL[("t", l)]] = float(np.asarray(inputs["t"][l]))
    params[:C, PCOL["linb"]] = inputs["lin_b"]
    params[:, PCOL["eps"]] = LN_EPS

    in_maps = []
    for k in range(NCORES):
        xs = np.ascontiguousarray(
            xp[k * NSH:(k + 1) * NSH].T).astype(bf16)
        gi = structs["gidx"][k]
        gw = np.tile(np.ascontiguousarray(gi.reshape(-1, 16).T), (8, 1))
        in_maps.append({
            "xt": xs, "encw": encw, "w1": w1, "w2": w2, "linw": linw,
            "ident": ident, "params": params, "gidx": gw,
            "rmat": structs["rmat"][k],
        })
    return in_maps


_BUILD_CACHE = {}


def _run(inputs, cfg=CFG, trace=False, tmpdir=None):
    import sys
    sys.path.insert(0, "/root/problem")
    from concourse.bass_utils import run_bass_kernel_spmd

    structs = build_edge_structs(inputs["edge_index"], cfg)
    nc, NPCOL, PCOL = build_nc(structs, cfg)
    in_maps = pack_inputs(inputs, structs, NPCOL, PCOL, cfg)
    res = run_bass_kernel_spmd(nc, in_maps, list(range(NCORES)), trace=trace,
                               tmpdir=tmpdir)
    outs = [res.results[k]["outp"] for k in range(NCORES)]  # [C, NSH] each
    full = np.concatenate(outs, axis=1).T                   # [NPAD, C]
    return np.ascontiguousarray(full[: cfg.N]).astype(np.float32), res


def kernel(**inputs) -> np.ndarray:
    out, _ = _run(inputs)
    return out

